# revision 15
# baseline (speedup 1.0000x reference)
"""Peephole-LSTM cell fused kernel for 8 Trainium2 NeuronCores.

Math (per reference):
    pre = X_t @ W + c_prev @ U + b          # W/U/b are the 4 gates concat'd
    f, i, o, c_hat = split(pre);  f,i,o = sigmoid;  c_hat = tanh
    c_t = f * c_prev + i * c_hat
    h_t = o * tanh(c_t)

Sharding: data-parallel over the batch dim (16384 -> 8 x 2048), weights
replicated, no cross-device communication.

Per-core device plan (B_loc=2048, D=512, 4H=2048, fp16 operands):
  - The kernel is PE-bound: 512 matmuls of [128,128]x[128,512] at
    ~217 ns each = 110.9 us floor.  Everything else is organised to
    keep the PE gapless from its first real matmul to its last.
  - Two HWDGE rings drain FIFO at combined ~358 GB/s; pieces are issued
    ungated in exact consumption order:
      sync:   X-h0 k0..3, W-h0 k0..3, C-h0 k0..3, U-h0 k0..3,
              cn0-5, X-h1 k0..3, C-h1 k0..3, cn6-15  (+ h stores)
      scalar: W-h1 k0..3, U-h1 k0..3
    The first real matmul needs only the first piece of each ring.
  - Junk matmuls (reading a scratch tile, only 128 cols memset) burn
    the HAM clock-gate ramp (~3.4 us @ 1.2 GHz) while the first pieces
    fly; they target phase-A banks so DCE keeps them.
  - Phase A: X@W only for batch tiles 0-7, one gate per pass
    (8 PSUM banks), k-outer so each arriving (X-k, W-k) piece pair is
    fully consumed before the next is needed (~293 GB/s demand).
    Partials staged to SBUF fp16 (PX) via DVE copies (ACT stays free).
  - Phase B staged (tiles 0-7): C@U accumulation + DVE add of PX.
    Phase B fused (tiles 8-15): 8-matmul PSUM accumulation.
    Elementwise split: ACT sigmoid/tanh from PSUM, DVE t2/ct or PX-add,
    GpSimd t1/h, so no engine exceeds the per-tile matmul budget.
  - Last tile: o-gate accumulated as two column groups [0:384]/[384:512]
    and f/i chains halved, so the exposed tail after the last matmul is
    one small sigmoid+multiply+DMA; the two h stores go to different
    rings.
"""

import sys

if "/opt/trn_rl_repo" not in sys.path:
    sys.path.insert(0, "/opt/trn_rl_repo")

import numpy as np

import concourse.bacc as bacc
import concourse.mybir as mybir
import concourse.tile as tile
from concourse import bass_utils

N_CORES = 8
B, D, H = 16384, 512, 512
BL = B // N_CORES          # 2048 rows per core
G4 = 4 * H                 # 2048, the concatenated gate dim
KT = D // 128              # 4 k-tiles
BT = BL // 128             # 16 batch tiles per core
JUNK_MMS = 8               # junk matmuls to lift the HAM clock gate
N_STAGED = 8               # batch tiles staged through PX (phase A)
PASS_ORDER = (3, 2, 1, 0)  # phase-A single-gate passes over tiles 0-7
STAGED_ORDER = (3, 2, 1, 0)  # staged-B per-tile gate order
BANK_ORDER = (3, 1, 0, 2)  # fused tiles: c_hat, i, f, o — deep chain first
LAST_ORDER = (3, 0, 1, 2)  # final tile: c_hat, f, i, o — o last, short tail
# Phase-B processing order: open with two fused tiles (gives the last
# phase-A eviction pass and the U quarters a 14 us buffer), then staged
# pairs separated by fused tiles so the heavier staged elementwise load
# never drifts far.
PHASE_B = (8, 9, 0, 1, 10, 2, 3, 11, 4, 5, 12, 6, 7, 13, 14, 15)

_cached = {}


def _build(has_bias: bool):
    key = has_bias
    if key in _cached:
        return _cached[key]

    f32 = mybir.dt.float32
    f16 = mybir.dt.float16
    AF = mybir.ActivationFunctionType
    Alu = mybir.AluOpType

    nc = bacc.Bacc("TRN2", target_bir_lowering=False, debug=False,
                   enable_asserts=False, enable_partition_id=False)
    xT = nc.dram_tensor("xT_f16", [D, BL], f16, kind="ExternalInput")
    cT = nc.dram_tensor("cT_f16", [D, BL], f16, kind="ExternalInput")
    c_f32 = nc.dram_tensor("c_f32", [BL, D], f32, kind="ExternalInput")
    w_f16 = nc.dram_tensor("w_f16", [D, G4], f16, kind="ExternalInput")
    u_f16 = nc.dram_tensor("u_f16", [D, G4], f16, kind="ExternalInput")
    if has_bias:
        bias_bc = nc.dram_tensor("bias_bc", [128, G4], f32, kind="ExternalInput")
    h_out = nc.dram_tensor("h_out", [BL, H], f32, kind="ExternalOutput")

    with tile.TileContext(nc) as tc:
        with (
            tc.tile_pool(name="const", bufs=1) as const,
            tc.tile_pool(name="px", bufs=1) as px_p,
            tc.tile_pool(name="psum", bufs=8, space="PSUM") as psum,
            tc.tile_pool(name="cnat", bufs=1) as cnat,
            tc.tile_pool(name="gates", bufs=8) as gate_p,
            tc.tile_pool(name="tmp1", bufs=3) as tmp1_p,
            tc.tile_pool(name="tmp2", bufs=3) as tmp2_p,
            tc.tile_pool(name="hsb", bufs=3) as h_p,
        ):
            # Operand pieces: X^T/C^T as [128, 1024] batch halves
            # (h0 = batch tiles 0-7); W/U as [128, 512] per-gate quarters.
            def piece(name):
                return const.tile([128, 1024], f16, tag=name, name=name)

            def qpiece(name):
                return const.tile([128, 512], f16, tag=name, name=name)

            XTh = [[piece(f"xt{k}h{q}") for q in range(2)] for k in range(KT)]
            CTh = [[piece(f"ct{k}h{q}") for q in range(2)] for k in range(KT)]
            Wq = [[qpiece(f"w{k}j{j}") for j in range(4)] for k in range(KT)]
            Uq = [[qpiece(f"u{k}j{j}") for j in range(4)] for k in range(KT)]

            CN = [cnat.tile([128, D], f32, tag=f"cn{bt}", name=f"cn{bt}")
                  for bt in range(BT)]

            def dram_h(t, k, half):
                return t.ap()[k * 128:(k + 1) * 128,
                              half * 1024:(half + 1) * 1024]

            def dram_q(t, k, jc):
                return t.ap()[k * 128:(k + 1) * 128,
                              jc * 512:(jc + 1) * 512]

            def cn_load(bt):
                nc.sync.dma_start(
                    out=CN[bt][:], in_=c_f32.ap()[bt * 128:(bt + 1) * 128, :]
                )

            # DMA issue: ungated, consumption order, two HWDGE rings
            # drained FIFO.  The k-cadence of phase A needs only the sync
            # ring's X pieces (~147 GB/s) plus one 128 KiB W quarter per
            # step from the scalar ring (~73 GB/s); C/U are not consumed
            # until phase B (~+32 us), so both rings have slack.
            for k in range(KT):
                nc.sync.dma_start(out=XTh[k][0][:], in_=dram_h(xT, k, 0))
            for k in range(KT):
                nc.sync.dma_start(out=CTh[k][0][:], in_=dram_h(cT, k, 0))
            for k in range(KT):
                nc.sync.dma_start(out=XTh[k][1][:], in_=dram_h(xT, k, 1))
            for k in range(KT):
                nc.sync.dma_start(out=CTh[k][1][:], in_=dram_h(cT, k, 1))
            for bt in (8, 9, 0, 1):
                cn_load(bt)
            for k in range(KT):
                nc.sync.dma_start(out=Uq[k][0][:], in_=dram_q(u_f16, k, 0))
            if has_bias:
                bias_sb = const.tile([128, G4], f32, tag="bias")
                nc.sync.dma_start(out=bias_sb[:], in_=bias_bc.ap())
            for bt in (10, 2, 3, 11, 4, 5, 12, 6, 7, 13, 14, 15):
                cn_load(bt)

            for jc in PASS_ORDER:
                for k in range(KT):
                    nc.scalar.dma_start(
                        out=Wq[k][jc][:], in_=dram_q(w_f16, k, jc)
                    )
            for jc in (3, 2, 1):
                for k in range(KT):
                    nc.scalar.dma_start(
                        out=Uq[k][jc][:], in_=dram_q(u_f16, k, jc)
                    )

            def lhsq(T, k, bt):
                q, r = divmod(bt, 8)
                return T[k][q][:, r * 128:(r + 1) * 128]

            # Junk warm-up: the HAM clock gate holds the PE at 1.2 GHz
            # until ~3.4 us of sustained activity; burn that window on
            # junk matmuls while the first operand pieces are in flight.
            # Only 128 cols of the scratch tile are memset (fast); the
            # rest streams garbage — results land in phase-A banks that
            # the first real (start=True) matmuls reset.
            junk = const.tile([128, 512], f16, tag="junk", name="junk")
            nc.gpsimd.memset(junk[:, 0:128], 0.0)

            PX = [px_p.tile([128, G4], f16, tag=f"px{bt}", name=f"px{bt}")
                  for bt in range(N_STAGED)]

            # Phase A: X@W only, one gate per pass over batch tiles 0-7
            # (8 PSUM banks), k-outer so each (X k-piece, W k-quarter)
            # pair is fully consumed before the next is needed.
            for pi, jc in enumerate(PASS_ORDER):
                ps8 = {
                    bt: psum.tile([128, 512], f32, tag="ps", name=f"psA{pi}_{bt}")
                    for bt in range(N_STAGED)
                }
                if pi == 0:
                    for j in range(JUNK_MMS):
                        nc.tensor.matmul(
                            ps8[j % N_STAGED][:], junk[:, 0:128], junk[:],
                            start=True, stop=True,
                        )
                for k in range(KT):
                    for bt in range(N_STAGED):
                        nc.tensor.matmul(
                            ps8[bt][:], lhsq(XTh, k, bt), Wq[k][jc][:],
                            start=(k == 0), stop=(k == KT - 1),
                        )
                jsl = slice(jc * 512, (jc + 1) * 512)
                for bt in range(N_STAGED):
                    nc.vector.tensor_copy(PX[bt][:, jsl], ps8[bt][:])

            # Phase B: per tile, fill the four gate banks and run the
            # eviction/elementwise chain bank-by-bank.
            for bt in PHASE_B:
                bsl = slice(bt * 128, (bt + 1) * 128)
                last = bt == BT - 1
                staged = bt < N_STAGED
                cn = CN[bt]

                if staged:
                    order = STAGED_ORDER
                elif last:
                    order = LAST_ORDER
                else:
                    order = BANK_ORDER
                gates = {}
                for jc in order:
                    jsl = slice(jc * 512, (jc + 1) * 512)
                    ps = psum.tile([128, 512], f32, tag="ps", name=f"psB{bt}_{jc}")
                    if staged:
                        for k in range(KT):
                            nc.tensor.matmul(
                                ps[:], lhsq(CTh, k, bt), Uq[k][jc][:],
                                start=(k == 0), stop=(k == KT - 1),
                            )
                        nc.vector.tensor_tensor(
                            ps[:], ps[:], PX[bt][:, jsl], Alu.add
                        )
                    elif last and jc == 2:
                        # Final o-gate: two column groups so the first
                        # half evicts while the second half finishes.
                        for csl in (slice(0, 256), slice(256, 512)):
                            for k in range(KT):
                                nc.tensor.matmul(
                                    ps[:, csl], lhsq(XTh, k, bt),
                                    Wq[k][jc][:, csl],
                                    start=(k == 0), stop=False,
                                )
                            for k in range(KT):
                                nc.tensor.matmul(
                                    ps[:, csl], lhsq(CTh, k, bt),
                                    Uq[k][jc][:, csl],
                                    start=False, stop=(k == KT - 1),
                                )
                    else:
                        for k in range(KT):
                            nc.tensor.matmul(
                                ps[:], lhsq(XTh, k, bt), Wq[k][jc][:],
                                start=(k == 0), stop=False,
                            )
                        for k in range(KT):
                            nc.tensor.matmul(
                                ps[:], lhsq(CTh, k, bt), Uq[k][jc][:],
                                start=False, stop=(k == KT - 1),
                            )
                    if has_bias:
                        nc.vector.tensor_tensor(
                            ps[:], ps[:], bias_sb[:, jsl], Alu.add
                        )
                    g = gate_p.tile([128, 512], f32, tag="g", name=f"g{bt}_{jc}")
                    if last and jc == 0:
                        nc.scalar.activation(g[:], ps[:], AF.Sigmoid)
                    elif last and jc == 1:
                        nc.scalar.activation(g[:], ps[:], AF.Sigmoid)
                    elif last and jc == 2:
                        for csl in (slice(0, 256), slice(256, 512)):
                            nc.scalar.activation(g[:, csl], ps[:, csl], AF.Sigmoid)
                    else:
                        nc.scalar.activation(
                            g[:], ps[:], AF.Tanh if jc == 3 else AF.Sigmoid
                        )
                    gates[jc] = g

                    # Chain steps as their inputs become ready.
                    if staged:
                        # order (3,2,1,0): after i compute t2; after f
                        # the rest of the chain (gpsimd) runs into the
                        # next tile's matmul window.
                        if jc == 1:
                            t2 = tmp2_p.tile([128, H], f32, tag="t2",
                                             name=f"t2_{bt}")
                            nc.vector.tensor_tensor(
                                t2[:], gates[1][:], gates[3][:], Alu.mult
                            )
                        elif jc == 0:
                            t1 = tmp1_p.tile([128, H], f32, tag="t1",
                                             name=f"t1_{bt}")
                            nc.gpsimd.tensor_tensor(
                                t1[:], gates[0][:], cn[:], Alu.mult
                            )
                            nc.gpsimd.tensor_tensor(t1[:], t1[:], t2[:], Alu.add)
                            tct = tmp2_p.tile([128, H], f32, tag="tct",
                                              name=f"tct{bt}")
                            nc.scalar.activation(tct[:], t1[:], AF.Tanh)
                            hsb = h_p.tile([128, H], f32, tag="h", name=f"h{bt}")
                            nc.gpsimd.tensor_tensor(
                                hsb[:], gates[2][:], tct[:], Alu.mult
                            )
                            nc.sync.dma_start(out=h_out.ap()[bsl, :], in_=hsb[:])
                    elif last:
                        # order (3,0,1,2), halved chains, o in two column
                        # groups with stores on separate rings.
                        if jc == 0:
                            t1 = tmp1_p.tile([128, H], f32, tag="t1",
                                             name=f"t1_{bt}")
                            for hs in range(2):
                                csl = slice(hs * 256, (hs + 1) * 256)
                                nc.gpsimd.tensor_tensor(
                                    t1[:, csl], gates[0][:, csl], cn[:, csl],
                                    Alu.mult,
                                )

                        elif jc == 1:
                            t2 = tmp2_p.tile([128, H], f32, tag="t2",
                                             name=f"t2_{bt}")
                            tct = tmp2_p.tile([128, H], f32, tag="tct",
                                              name=f"tct{bt}")
                            for hs in range(2):
                                csl = slice(hs * 256, (hs + 1) * 256)
                                nc.vector.tensor_tensor(
                                    t2[:, csl], gates[1][:, csl],
                                    gates[3][:, csl], Alu.mult,
                                )
                                nc.vector.tensor_tensor(
                                    t1[:, csl], t1[:, csl], t2[:, csl], Alu.add
                                )
                                nc.scalar.activation(
                                    tct[:, csl], t1[:, csl], AF.Tanh
                                )
                        elif jc == 2:
                            hsb = h_p.tile([128, H], f32, tag="h", name=f"h{bt}")
                            csl = slice(0, 256)
                            nc.vector.tensor_tensor(
                                hsb[:, csl], gates[2][:, csl], tct[:, csl],
                                Alu.mult,
                            )
                            nc.sync.dma_start(
                                out=h_out.ap()[bsl, csl], in_=hsb[:, csl]
                            )
                            csl = slice(256, 512)
                            nc.vector.tensor_tensor(
                                hsb[:, csl], gates[2][:, csl], tct[:, csl],
                                Alu.mult,
                            )
                            nc.scalar.dma_start(
                                out=h_out.ap()[bsl, csl], in_=hsb[:, csl]
                            )
                    else:
                        # fused tiles 8-14, order (3,1,0,2)
                        if jc == 1:
                            t2 = tmp2_p.tile([128, H], f32, tag="t2",
                                             name=f"t2_{bt}")
                            nc.vector.tensor_tensor(
                                t2[:], gates[1][:], gates[3][:], Alu.mult
                            )
                        elif jc == 0:
                            t1 = tmp1_p.tile([128, H], f32, tag="t1",
                                             name=f"t1_{bt}")
                            nc.gpsimd.tensor_tensor(
                                t1[:], gates[0][:], cn[:], Alu.mult
                            )
                            nc.vector.tensor_tensor(t1[:], t1[:], t2[:], Alu.add)
                            tct = tmp2_p.tile([128, H], f32, tag="tct",
                                              name=f"tct{bt}")
                            nc.scalar.activation(tct[:], t1[:], AF.Tanh)
                        elif jc == 2:
                            hsb = h_p.tile([128, H], f32, tag="h", name=f"h{bt}")
                            nc.gpsimd.tensor_tensor(
                                hsb[:], gates[2][:], tct[:], Alu.mult
                            )
                            nc.sync.dma_start(out=h_out.ap()[bsl, :], in_=hsb[:])

    nc.compile()
    _cached[key] = nc
    return nc


def _prep(X_t, c_prev, W_f, W_i, W_o, W_c, U_f, U_i, U_o, U_c, b_f, b_i, b_o, b_c):
    """Host-side (free) preprocessing: concat, cast, transpose, shard."""
    f16 = np.float16
    W = np.ascontiguousarray(
        np.concatenate([W_f, W_i, W_o, W_c], axis=1).astype(f16)
    )
    U = np.ascontiguousarray(
        np.concatenate([U_f, U_i, U_o, U_c], axis=1).astype(f16)
    )
    b = np.concatenate([b_f, b_i, b_o, b_c], axis=0).astype(np.float32)
    has_bias = bool(np.any(b != 0.0))

    X16 = np.asarray(X_t).astype(f16)
    C16 = np.asarray(c_prev).astype(f16)
    C32 = np.asarray(c_prev).astype(np.float32)

    in_maps = []
    for i in range(N_CORES):
        sl = slice(i * BL, (i + 1) * BL)
        m = {
            "xT_f16": np.ascontiguousarray(X16[sl].T),
            "cT_f16": np.ascontiguousarray(C16[sl].T),
            "c_f32": np.ascontiguousarray(C32[sl]),
            "w_f16": W,
            "u_f16": U,
        }
        if has_bias:
            m["bias_bc"] = np.ascontiguousarray(
                np.broadcast_to(b[None, :], (128, G4))
            )
        in_maps.append(m)
    return in_maps, has_bias


def kernel(**inputs):
    in_maps, has_bias = _prep(**inputs)
    nc = _build(has_bias)
    last_err = None
    for _ in range(3):
        try:
            res = bass_utils.run_bass_kernel_spmd(
                nc, in_maps, core_ids=list(range(N_CORES))
            )
            break
        except Exception as e:  # intermittent device wedge: retry
            last_err = e
            import time
            time.sleep(5)
    else:
        raise last_err
    return np.concatenate([res.results[i]["h_out"] for i in range(N_CORES)], axis=0)
